# revision 1
# baseline (speedup 1.0000x reference)
"""BiMamba block Trainium2 kernel.

Sharding: pure data-parallel over (direction, batch) = 2*4 = 8 units, one per
NeuronCore. Each core runs an identical Bass program computing a full Mamba
forward pass for one (batch, direction) sequence:

    xz = in_proj @ x            (PE, fp16)
    xc = silu(causal_conv4(xi)) (DVE taps reading PSUM + ACT silu)
    proj = x_proj @ xc          (PE)  -> dt_raw[64], B[16], C[16]
    dt = softplus(dt_proj @ dt_raw + dtb)  (PE + ACT)
    h[d,n,t] = exp(A[d,n]dt[d,t]) * h[d,n,t-1] + dt*xc*B   (ACT exp with
        per-partition scale=A fusing the A-multiply; DVE tensor_tensor_scan
        over time, fp32 decay; most u-multiplies offloaded to GPSIMD)
    ys = sum_n C*h   (DVE mult + PE identity-matmul PSUM accumulation)
    y  = out_proj @ ((ys + xc*Dp) * silu(z))   (PE)

Host flips the time axis for the backward direction before/after, and sums
the two directions' outputs.

Layout on device: channels on partitions (16 blocks of 128), time on the free
axis.  All 16-bit tensors are fp16 (PE runs fp16 at full rate; 5x smaller
rounding error than bf16). The scan decay dA stays fp32.
"""

from contextlib import ExitStack

import numpy as np

D_MODEL, D_STATE, D_CONV = 1024, 16, 4
D_INNER = 2048
DT_RANK = 64
B_SZ, SEQ = 4, 2048
NB = D_INNER // 128  # 16 channel blocks
NT = SEQ // 512      # 4 psum column tiles
U_DVE_N = 5          # u-mults with n < this on DVE; rest GPSIMD

_CACHE = {}


def _pack_consts(conv_w, conv_b, dtb, Dp, A):
    # [128, 16*23] f32; per blk: A(16)|cw(4)|cb|dtb|Dp, rows = channel%128
    out = np.zeros((128, 16 * 23), np.float32)
    for blk in range(16):
        sl = slice(blk * 128, (blk + 1) * 128)
        out[:, blk * 23:blk * 23 + 16] = A[sl]
        out[:, blk * 23 + 16:blk * 23 + 20] = conv_w[sl]
        out[:, blk * 23 + 20] = conv_b[sl]
        out[:, blk * 23 + 21] = dtb[sl]
        out[:, blk * 23 + 22] = Dp[sl]
    return out



def _pad_xwT(xw):
    # xw: [96, 2048] -> transpose and pad to [2048, 112] with C at cols 96:112
    out = np.zeros((2048, 112), np.float16)
    xwT = xw.T.astype(np.float16)
    out[:, 0:80] = xwT[:, 0:80]
    out[:, 96:112] = xwT[:, 80:96]
    return out



def build_program():
    import concourse.bass as bass
    import concourse.bacc as bacc
    import concourse.tile as tile
    from concourse import mybir
    from concourse.masks import make_identity

    f16 = mybir.dt.float16
    f32 = mybir.dt.float32
    AF = mybir.ActivationFunctionType
    OP = mybir.AluOpType

    nc = bacc.Bacc()

    xT = nc.declare_dram_parameter("xT", [D_MODEL, SEQ], f16, isOutput=False)
    in_wT = nc.declare_dram_parameter("in_wT", [D_MODEL, 2 * D_INNER], f16, isOutput=False)
    XPW = 112  # x_proj out: dt_raw 0:64, B 64:80, pad, C 96:112 (32-part alignment)
    xwT = nc.declare_dram_parameter("xwT", [D_INNER, XPW], f16, isOutput=False)
    dtwT = nc.declare_dram_parameter("dtwT", [DT_RANK, D_INNER], f16, isOutput=False)
    owT = nc.declare_dram_parameter("owT", [D_INNER, D_MODEL], f16, isOutput=False)
    # per-channel consts packed host-side: per blk 23 cols = A(16)|cw(4)|cb|dtb|Dp
    CPW = 23
    consts_d = nc.declare_dram_parameter("consts_packed", [128, NB * CPW], f32, isOutput=False)
    y_out = nc.declare_dram_parameter("y", [D_MODEL, SEQ], f32, isOutput=True)

    # DRAM staging (spills)
    xc_d = nc.dram_tensor("xc_d", [D_INNER, SEQ], f16)
    sz_d = nc.dram_tensor("sz_d", [D_INNER, SEQ], f16)
    yf_d = nc.dram_tensor("yf_d", [D_INNER, SEQ], f16)
    B_d = nc.dram_tensor("B_d", [D_STATE, SEQ], f16)
    C_d = nc.dram_tensor("C_d", [D_STATE, SEQ], f16)

    with tile.TileContext(nc) as tc, ExitStack() as ctx:
        consts = ctx.enter_context(tc.tile_pool(name="consts", bufs=1))

        # ---- constants ----
        I128 = consts.tile([128, 128], f16, tag="I128")
        make_identity(nc, I128)
        call = consts.tile([128, NB * CPW], f32, tag="call")
        nc.sync.dma_start(out=call, in_=consts_d[:, :])
        def A_col(blk, n):
            return call[:, blk * CPW + n:blk * CPW + n + 1]
        def cw_col(blk, k):
            return call[:, blk * CPW + 16 + k:blk * CPW + 16 + k + 1]
        def cb_col(blk):
            return call[:, blk * CPW + 20:blk * CPW + 21]
        def dtb_col(blk):
            return call[:, blk * CPW + 21:blk * CPW + 22]
        def Dp_col(blk):
            return call[:, blk * CPW + 22:blk * CPW + 23]
        xwT_sb = []
        xcpool0 = tc.alloc_tile_pool(name="xwpool", bufs=1)  # released after P3
        for k in range(NB):
            t = xcpool0.tile([128, XPW], f16, tag=f"xwT{k}", name=f"xwT{k}")
            nc.sync.dma_start(out=t, in_=xwT[k * 128:(k + 1) * 128, :])
            xwT_sb.append(t)
        dtwT_sb = consts.tile([DT_RANK, D_INNER], f16, tag="dtwT")
        nc.sync.dma_start(out=dtwT_sb, in_=dtwT[:, :])
        # dt_raw persists P3->P4
        dtraw = consts.tile([DT_RANK, SEQ], f16, tag="dtraw")

        # ---- P1: in_proj + conv/silu (xi) + silu (z) ----
        xcpool = tc.alloc_tile_pool(name="xcpool", bufs=1)  # released after P3
        xc_sb = [None] * NB
        with tc.tile_pool(name="p1", bufs=1) as p1pool, \
             tc.tile_pool(name="p1w", bufs=2) as p1w, \
             tc.tile_pool(name="pz", bufs=2, space="PSUM") as pz:
            xT_sb = []
            for k in range(8):
                t = p1pool.tile([128, SEQ], f16, tag=f"xT{k}")
                nc.sync.dma_start(out=t, in_=xT[k * 128:(k + 1) * 128, :])
                xT_sb.append(t)
            inw_sb = []
            for k in range(8):
                t = p1pool.tile([128, 2 * D_INNER], f16, tag=f"inw{k}")
                nc.sync.dma_start(out=t, in_=in_wT[k * 128:(k + 1) * 128, :])
                inw_sb.append(t)

            for m in range(32):
                psum = pz.tile([128, SEQ], f32, tag="xz")
                for nt in range(NT):
                    cs = slice(nt * 512, (nt + 1) * 512)
                    for k in range(8):
                        nc.tensor.matmul(
                            psum[:, cs],
                            lhsT=inw_sb[k][:, m * 128:(m + 1) * 128],
                            rhs=xT_sb[k][:, cs],
                            start=(k == 0), stop=(k == 7),
                        )
                if m < NB:
                    blk = m
                    # stage xi in SBUF (per-bank copies keep sync fanin low)
                    xi_t = p1w.tile([128, SEQ], f16, tag="xi")
                    for nt in range(NT // 2):
                        cs = slice(nt * 1024, (nt + 1) * 1024)
                        nc.scalar.activation(out=xi_t[:, cs], in_=psum[:, cs], func=AF.Copy)
                    acc = p1w.tile([128, SEQ], f16, tag="acc")
                    # tap 3 + bias
                    nc.vector.tensor_scalar(
                        out=acc, in0=xi_t,
                        scalar1=cw_col(blk, 3),
                        scalar2=cb_col(blk),
                        op0=OP.mult, op1=OP.add,
                    )
                    for k in range(3):
                        d = 3 - k
                        nc.vector.scalar_tensor_tensor(
                            out=acc[:, d:], in0=xi_t[:, :SEQ - d],
                            scalar=cw_col(blk, k),
                            in1=acc[:, d:], op0=OP.mult, op1=OP.add,
                        )
                    xc_t = xcpool.tile([128, SEQ], f16, tag=f"xc{blk}", name=f"xc{blk}")
                    xc_sb[blk] = xc_t
                    nc.scalar.activation(out=xc_t, in_=acc, func=AF.Silu)
                    nc.sync.dma_start(out=xc_d[blk * 128:(blk + 1) * 128, :], in_=xc_t)
                else:
                    blk = m - NB
                    sz_t = p1w.tile([128, SEQ], f16, tag="sz")
                    for nt in range(NT):
                        cs = slice(nt * 512, (nt + 1) * 512)
                        nc.scalar.activation(out=sz_t[:, cs], in_=psum[:, cs], func=AF.Silu)
                    nc.sync.dma_start(out=sz_d[blk * 128:(blk + 1) * 128, :], in_=sz_t)

        # ---- P3: x_proj ----
        with tc.tile_pool(name="p3", bufs=3) as p3pool, \
             tc.tile_pool(name="pp3", bufs=1, space="PSUM") as pp3:
            psum_proj = pp3.tile([XPW, SEQ], f32, tag="proj")
            for nt in range(NT):
                cs = slice(nt * 512, (nt + 1) * 512)
                for k in range(NB):
                    nc.tensor.matmul(
                        psum_proj[:, cs], lhsT=xwT_sb[k], rhs=xc_sb[k][:, cs],
                        start=(k == 0), stop=(k == NB - 1),
                    )
            B_sb = p3pool.tile([D_STATE, SEQ], f16, tag="Bs")
            C_sb = p3pool.tile([D_STATE, SEQ], f16, tag="Cs")
            for nt in range(NT // 2):
                cs = slice(nt * 1024, (nt + 1) * 1024)
                nc.scalar.activation(out=dtraw[:, cs], in_=psum_proj[0:DT_RANK, cs], func=AF.Copy)
                nc.scalar.activation(out=B_sb[:, cs], in_=psum_proj[64:80, cs], func=AF.Copy)
                nc.scalar.activation(out=C_sb[:, cs], in_=psum_proj[96:112, cs], func=AF.Copy)
            nc.sync.dma_start(out=B_d[:, :], in_=B_sb)
            nc.sync.dma_start(out=C_d[:, :], in_=C_sb)
        xcpool.release()
        xcpool0.release()

        # ---- P4+P5 fused, two time-halves: halved tiles allow deeper
        # buffering (bufs 3-4) within SBUF; dt is produced inline during
        # half 0 and kept resident for half 1. Scan state carries across
        # halves via carry_all. ----
        HL = SEQ // 2
        NTH = HL // 512
        carry_all = consts.tile([128, NB * D_STATE], f16, tag="carry_all")
        dtpool = tc.alloc_tile_pool(name="dtpool", bufs=1)
        dt_sb = [None] * NB
        with tc.tile_pool(name="bc", bufs=1) as bc_pool, \
             tc.tile_pool(name="p5s", bufs=3) as p5s, \
             tc.tile_pool(name="p5w", bufs=4) as p5w, \
             tc.tile_pool(name="p5dA", bufs=4) as p5dA, \
             tc.tile_pool(name="ppy", bufs=3, space="PSUM") as ppy, \
             tc.tile_pool(name="ppdt", bufs=1, space="PSUM") as ppdt:
            for half in range(2):
                hs = slice(half * HL, (half + 1) * HL)
                B_bc = bc_pool.tile([128, D_STATE * HL], f16, tag="B_bc")
                C_bc = bc_pool.tile([128, D_STATE * HL], f16, tag="C_bc")
                B_src = bass.AP(tensor=B_d, offset=half * HL,
                                ap=[[0, 128], [SEQ, D_STATE], [1, HL]])
                C_src = bass.AP(tensor=C_d, offset=half * HL,
                                ap=[[0, 128], [SEQ, D_STATE], [1, HL]])
                nc.sync.dma_start(out=B_bc, in_=B_src)
                nc.sync.dma_start(out=C_bc, in_=C_src)
                for blk in range(NB):
                    rs = slice(blk * 128, (blk + 1) * 128)
                    if half == 0:
                        # dt_proj + softplus(v+dtb) = ln(1+exp(v+dtb)), full L
                        dt_full = dtpool.tile([128, SEQ], f16, tag=f"dt{blk}",
                                              name=f"dt{blk}")
                        dt_sb[blk] = dt_full
                        for nt in range(NT // 2):
                            cs = slice(nt * 1024, (nt + 1) * 1024)
                            psum_dt = ppdt.tile([128, 1024], f32, tag="pdt",
                                                name=f"pdt{blk}_{nt}")
                            for sb in range(2):
                                ss = slice(sb * 512, (sb + 1) * 512)
                                nc.tensor.matmul(
                                    psum_dt[:, ss],
                                    lhsT=dtwT_sb[:, blk * 128:(blk + 1) * 128],
                                    rhs=dtraw[:, nt * 1024 + sb * 512:
                                              nt * 1024 + (sb + 1) * 512],
                                    start=True, stop=True,
                                )
                            nc.scalar.activation(
                                out=psum_dt, in_=psum_dt, func=AF.Exp,
                                bias=dtb_col(blk), scale=1.0,
                            )
                            nc.scalar.activation(
                                out=dt_full[:, cs], in_=psum_dt, func=AF.Ln,
                                bias=1.0, scale=1.0)
                    dt_t = dt_sb[blk][:, hs]
                    xc_t = p5s.tile([128, HL], f16, tag="xcs2")
                    nc.sync.dma_start(out=xc_t, in_=xc_d[rs, hs])
                    sz_t = p5s.tile([128, HL], f16, tag="szs")
                    nc.sync.dma_start(out=sz_t, in_=sz_d[rs, hs])
                    dtxc = p5w.tile([128, HL], f16, tag="dtxc")
                    nc.gpsimd.tensor_mul(out=dtxc, in0=dt_t, in1=xc_t)
                    psum_y = ppy.tile([128, HL], f32, tag="py")
                    for n in range(D_STATE):
                        ns = slice(n * HL, (n + 1) * HL)
                        cc = blk * D_STATE + n
                        dA = p5dA.tile([128, HL], f32, tag="dA")
                        nc.scalar.activation(
                            out=dA, in_=dt_t, func=AF.Exp,
                            scale=A_col(blk, n),
                        )
                        u = p5w.tile([128, HL], f16, tag="u")
                        u_eng = nc.vector if n < U_DVE_N else nc.gpsimd
                        u_eng.tensor_mul(out=u, in0=dtxc, in1=B_bc[:, ns])
                        h = p5w.tile([128, HL], f16, tag="h", bufs=5)
                        init = 0.0 if half == 0 else carry_all[:, cc:cc + 1]
                        nc.vector.tensor_tensor_scan(
                            out=h, data0=dA, data1=u, initial=init,
                            op0=OP.mult, op1=OP.add,
                        )
                        if half == 0:
                            nc.vector.tensor_copy(
                                out=carry_all[:, cc:cc + 1], in_=h[:, HL - 1:HL])
                        hc = p5w.tile([128, HL], f16, tag="hc")
                        nc.vector.tensor_mul(out=hc, in0=h, in1=C_bc[:, ns])
                        for nt in range(NTH):
                            cs = slice(nt * 512, (nt + 1) * 512)
                            nc.tensor.matmul(
                                psum_y[:, cs], lhsT=I128, rhs=hc[:, cs],
                                start=(n == 0), stop=(n == D_STATE - 1),
                            )
                    y1 = p5w.tile([128, HL], f16, tag="dtxc", name=f"y1_{half}_{blk}")
                    for nt in range(NTH // 2):
                        cs = slice(nt * 1024, (nt + 1) * 1024)
                        nc.vector.scalar_tensor_tensor(
                            out=y1[:, cs], in0=xc_t[:, cs], scalar=Dp_col(blk),
                            in1=psum_y[:, cs], op0=OP.mult, op1=OP.add,
                        )
                    yf = p5w.tile([128, HL], f16, tag="u", name=f"yf_{half}_{blk}")
                    nc.gpsimd.tensor_mul(out=yf, in0=y1, in1=sz_t)
                    nc.sync.dma_start(out=yf_d[rs, hs], in_=yf)
        dtpool.release()

        # ---- P6: out_proj ----
        with tc.tile_pool(name="p6w", bufs=1) as p6w, \
             tc.tile_pool(name="p6", bufs=6) as p6pool, \
             tc.tile_pool(name="ppo", bufs=1, space="PSUM") as ppo:
            owT_sb = []
            for k in range(NB):
                t = p6w.tile([128, D_MODEL], f16, tag=f"owT{k}", name=f"owT{k}")
                nc.sync.dma_start(out=t, in_=owT[k * 128:(k + 1) * 128, :])
                owT_sb.append(t)
            for nt in range(NT):
                cs = slice(nt * 512, (nt + 1) * 512)
                psum_o = [ppo.tile([128, 512], f32, tag=f"po{m}", name=f"po{m}_{nt}") for m in range(8)]
                for k in range(NB):
                    rt = p6pool.tile([128, 512], f16, tag="yfs")
                    nc.sync.dma_start(out=rt, in_=yf_d[k * 128:(k + 1) * 128, cs])
                    for m in range(8):
                        nc.tensor.matmul(
                            psum_o[m],
                            lhsT=owT_sb[k][:, m * 128:(m + 1) * 128],
                            rhs=rt, start=(k == 0), stop=(k == NB - 1),
                        )
                for m in range(8):
                    yo = p6pool.tile([128, 512], f32, tag="yo")
                    nc.scalar.activation(out=yo, in_=psum_o[m], func=AF.Copy)
                    nc.sync.dma_start(out=y_out[m * 128:(m + 1) * 128, cs], in_=yo)

    nc.finalize()
    return nc


def _get_nc():
    if "nc" not in _CACHE:
        _CACHE["nc"] = build_program()
    return _CACHE["nc"]


def kernel(x, in_proj_w, conv_w, conv_b, x_proj_w, dt_proj_w, dt_proj_b,
           A_log, D_param, out_proj_w):
    from concourse.bass_utils import run_bass_kernel_spmd

    nc = _get_nc()

    x = np.asarray(x)
    wk = {}
    for d in range(2):
        wk[d] = {
            "in_wT": np.ascontiguousarray(np.asarray(in_proj_w[d]).T).astype(np.float16),
            "xwT": _pad_xwT(np.asarray(x_proj_w[d])),
            "dtwT": np.ascontiguousarray(np.asarray(dt_proj_w[d]).T).astype(np.float16),
            "owT": np.ascontiguousarray(np.asarray(out_proj_w[d]).T).astype(np.float16),
            "consts_packed": _pack_consts(
                np.asarray(conv_w[d]).astype(np.float32),
                np.asarray(conv_b[d]).astype(np.float32),
                np.asarray(dt_proj_b[d]).astype(np.float32),
                np.asarray(D_param[d]).astype(np.float32),
                (-np.exp(np.asarray(A_log[d]))).astype(np.float32)),
        }

    in_maps = []
    for u in range(8):
        d, b = divmod(u, 4)
        xb = np.asarray(x[b])
        if d == 1:
            xb = xb[::-1]
        m = dict(wk[d])
        m["xT"] = np.ascontiguousarray(xb.T).astype(np.float16)
        in_maps.append(m)

    res = run_bass_kernel_spmd(nc, in_maps, core_ids=list(range(8))).results

    out = np.zeros((B_SZ, SEQ, D_MODEL), np.float32)
    for u in range(8):
        d, b = divmod(u, 4)
        yu = res[u]["y"].T  # [SEQ, D_MODEL]
        if d == 1:
            yu = yu[::-1]
        out[b] += yu
    return out.astype(np.float32)

